# revision 34
# baseline (speedup 1.0000x reference)
"""LlamaAttention (B=1, S=2048, D=2048, H=16, KV=4) on 8 TRN2 NeuronCores.

Tensor-parallel over heads: core c owns q-heads [2c, 2c+1] and kv-head c//2.
Each core computes partial = attn_out_c @ Wo[:, c-slice].T over the full
sequence; the all-reduce after o_proj happens on the host (sum of partials).

Layout strategy: everything on-chip lives feature-on-partitions ("transposed"):
  hsT [d, s], qT/kT/vT [hd, s], attn_outT [hd, s].  The host pre-transposes
hidden_states and weights into partition-major [128, N] bf16 arrays so every
DMA is contiguous; rope tables (bf16 cos / sign-adjusted sin) and the causal
diagonal mask block (transposed) are precomputed on host.

Key design point vs a classic flash-style schedule: attention scores are
computed TRANSPOSED (s^T[k, q] via stationary = krot k-tile, moving = qrot
q-columns), so the exp writes P^T [k-on-partitions] straight into SBUF and
P@V consumes it directly as the moving operand.  No DMA xbar transposes of P
are needed (each DMA transpose acts as a full DMA-subsystem barrier on TRN2,
which serialized the previous design).  The softmax row sums l[q] are
accumulated with an all-ones stationary matmul into a PSUM tile alongside
the P@V accumulation (every output partition holds the same l row), so the
aout drain is one reciprocal + one multiply, with no cross-partition
broadcast.

Schedule (all matmuls bf16):
 - QKV projects per quarter in TWO 2-bank passes (q-heads, then k/v), so
   PSUM fits: 2 qkv + 2 scores + 1 l + 1 pv + 2 o_proj = 8 banks.
 - attention chunk c (scores c for all k-tiles j<=4c+3, l+PV for units
   g=c, o_proj group c) interleaves into quarter c+1's projection stream;
   chunk 3 forms the tail as a per-j software pipeline
   [score MM j+1 | exp j | l/PV MM j] so the PE never waits a full exp.
 - softmax: plain exp (no running max: scores are O(6) sigma so fp32 exp
   cannot overflow); P^T stays unnormalized bf16, aout = pv * (1/l).
 - PSUM drains: rope uses one scalar cast then all-bf16 DVE ops (the
   rotate-half is a partition-shifted DVE copy); o_proj casts alternate
   scalar/vector.  Output partials are bf16, host all-reduces in f32.
"""
import math
import numpy as np

S = 2048
D = 2048
HD = 128
H = 16
KV = 4
NCORES = 8
NT = S // 128          # 16 sequence tiles
DTC = D // 128         # 16 feature chunks
QH = H // NCORES       # 2 q-heads per core
ROPE_BASE = 10000.0
SCALE = 1.0 / math.sqrt(HD)
NEG = -1.0e9

_CACHE = {}


def _pt_layout(c):
    """Column layout of the P^T chunk-c buffer: per k-tile j the slice
    (offset, width) covering q-columns [max(512c, 128j), 512c+512)."""
    offs = []
    off = 0
    for j in range(4 * c + 4):
        w = min(512, 512 * c + 512 - 128 * j)
        offs.append((off, w))
        off += w
    return offs, off


def _rope(nc, pool, dst, src_ps, cos_sb, sin_sb, cols, BF16, ALU):
    """dst[:, cols] = src*cos + rotate_half(src)*sin  (src: psum [128, w])."""
    w = cols.stop - cols.start
    raw = pool.tile([128, w], BF16, tag="roperaw")
    rot = pool.tile([128, w], BF16, tag="roperot")
    t1 = pool.tile([128, w], BF16, tag="ropet1")
    nc.scalar.copy(out=raw, in_=src_ps)
    nc.vector.tensor_copy(out=rot[0:64, :], in_=raw[64:128, :])
    nc.vector.tensor_copy(out=rot[64:128, :], in_=raw[0:64, :])
    nc.vector.tensor_tensor(out=t1, in0=raw, in1=cos_sb[:, cols], op=ALU.mult)
    nc.vector.tensor_tensor(out=rot, in0=rot, in1=sin_sb[:, cols], op=ALU.mult)
    nc.vector.tensor_tensor(out=dst[:, cols], in0=t1, in1=rot, op=ALU.add)


def build_nc():
    import concourse.bacc as bacc
    import concourse.tile as tile
    from concourse import mybir

    F32 = mybir.dt.float32
    BF16 = mybir.dt.bfloat16
    AF = mybir.ActivationFunctionType
    ALU = mybir.AluOpType

    nc = bacc.Bacc("TRN2", target_bir_lowering=False, debug=False)
    hs_d = nc.dram_tensor("hs", [128, DTC * S], BF16, kind="ExternalInput").ap()
    wq_d = nc.dram_tensor("wq", [128, DTC * QH * 128], BF16, kind="ExternalInput").ap()
    wk_d = nc.dram_tensor("wk", [128, DTC * 128], BF16, kind="ExternalInput").ap()
    wv_d = nc.dram_tensor("wv", [128, DTC * 128], BF16, kind="ExternalInput").ap()
    wo_d = nc.dram_tensor("wo", [128, QH * D], BF16, kind="ExternalInput").ap()
    cos_d = nc.dram_tensor("cos", [128, S], BF16, kind="ExternalInput").ap()
    sin_d = nc.dram_tensor("sin", [128, S], BF16, kind="ExternalInput").ap()
    tri_d = nc.dram_tensor("tri", [128, 128], F32, kind="ExternalInput").ap()
    out_d = nc.dram_tensor("out", [128, NT * D], BF16, kind="ExternalOutput").ap()

    hs3 = hs_d.rearrange("p (t s) -> p t s", t=DTC)
    out3 = out_d.rearrange("p (t d) -> p t d", t=NT)

    HALF = S // 2
    QTR = S // 4

    with tile.TileContext(nc) as tc:
        with tc.tile_pool(name="consts", bufs=1) as consts, \
             tc.tile_pool(name="persist", bufs=1) as persist, \
             tc.tile_pool(name="hsp", bufs=8) as hsp, \
             tc.tile_pool(name="ropet", bufs=1) as ropet, \
             tc.tile_pool(name="lbp", bufs=2) as lbp, \
             tc.tile_pool(name="osb", bufs=4) as osb, \
             tc.tile_pool(name="osb0", bufs=4) as osb0p, \
             tc.tile_pool(name="qkvps", bufs=1, space="PSUM") as qkvps, \
             tc.tile_pool(name="sps", bufs=2, space="PSUM") as sps, \
             tc.tile_pool(name="lps", bufs=1, space="PSUM") as lpsp, \
             tc.tile_pool(name="pvps", bufs=1, space="PSUM") as pvps, \
             tc.tile_pool(name="pops", bufs=2, space="PSUM") as pops:
            tri_sb = consts.tile([128, 128], F32)
            ones_sb = consts.tile([128, 128], BF16)
            cos_sb = consts.tile([128, S], BF16)
            sin_sb = consts.tile([128, S], BF16)
            wq_sb = consts.tile([128, DTC, QH * 128], BF16)
            wk_sb = consts.tile([128, DTC, 128], BF16)
            wv_sb = consts.tile([128, DTC, 128], BF16)
            wo_sb = consts.tile([128, QH, D], BF16)

            qrot = [persist.tile([128, S], BF16, tag=f"qrot{h}", name=f"qrot{h}") for h in range(QH)]
            krot = persist.tile([128, S], BF16, tag="krot")
            vbf = persist.tile([128, S], BF16, tag="vbf")
            vnat = persist.tile([128, NT * 128], BF16, tag="vnat")
            vnat3 = vnat.rearrange("p (t f) -> p t f", t=NT)
            aout = [persist.tile([128, S], BF16, tag=f"aout{h}", name=f"aout{h}") for h in range(QH)]
            # P^T chunk buffers: pt[h][c] holds exp(s^T) for q-chunk c,
            # k-tiles j=0..4c+3 consecutively (see _pt_layout).
            pt_offs = {}
            pt = [[None] * 4 for _ in range(QH)]
            for h in range(QH):
                for c in range(4):
                    offs, L = _pt_layout(c)
                    pt_offs[c] = offs
                    pt[h][c] = persist.tile([128, L], BF16, tag=f"pt{h}_{c}",
                                            name=f"pt{h}_{c}")

            nc.vector.memset(ones_sb, 1.0)

            hst_tiles = {}

            def load_hst(sh, j):
                t = hsp.tile([128, 2, HALF], BF16, tag="hst", name=f"hst{sh}_{j}")
                nc.sync.dma_start(
                    out=t, in_=hs3[:, 2 * j:2 * j + 2, sh * HALF:(sh + 1) * HALF])
                hst_tiles[(sh, j)] = t
                return t

            # DMA prologue (all on the sync ring): first weight chunks + hs
            # tiles ahead of the bulk so the PE starts within ~3us.
            wq3 = wq_d.rearrange("p (t m) -> p t m", t=DTC)
            wk3 = wk_d.rearrange("p (t m) -> p t m", t=DTC)
            wv3 = wv_d.rearrange("p (t m) -> p t m", t=DTC)

            # pass A consumes only wq, so stream all of wq first, then wk/wv
            # (needed ~8us later by pass B), interleaved with hs tiles.
            nc.sync.dma_start(out=tri_sb, in_=tri_d)
            nc.sync.dma_start(out=wq_sb[:, 0:4, :], in_=wq3[:, 0:4, :])
            load_hst(0, 0)
            load_hst(0, 1)
            nc.sync.dma_start(out=wq_sb[:, 4:10, :], in_=wq3[:, 4:10, :])
            load_hst(0, 2)
            nc.sync.dma_start(out=wq_sb[:, 10:16, :], in_=wq3[:, 10:16, :])
            load_hst(0, 3)
            nc.sync.dma_start(out=cos_sb, in_=cos_d)
            nc.sync.dma_start(out=sin_sb, in_=sin_d)
            nc.sync.dma_start(out=wk_sb, in_=wk3)
            nc.sync.dma_start(out=wv_sb, in_=wv3)
            for j in range(4, 8):
                load_hst(0, j)
            nc.sync.dma_start(out=wo_sb, in_=wo_d.rearrange("p (h m) -> p h m", h=QH))

            # ---------------- generators --------------------------------
            def qkv_gen(qtr):
                """Quarter qtr of the QKV projection in two 2-bank passes."""
                sh, qq = divmod(qtr, 2)
                cols = slice(qtr * QTR, (qtr + 1) * QTR)
                if qtr == 1:
                    for j in range(8):   # prefetch half 1 as slots free up
                        load_hst(1, j)
                # pass A: the two q heads
                pqa = [qkvps.tile([128, QTR], F32, tag=f"qk{m}", name=f"pqa{qtr}_{m}")
                       for m in range(QH)]
                for j in range(DTC // 2):
                    hst = hst_tiles[(sh, j)]
                    for t2 in range(2):
                        dt = 2 * j + t2
                        for m in range(QH):
                            nc.tensor.matmul(pqa[m], wq_sb[:, dt, m * 128:(m + 1) * 128],
                                             hst[:, t2, qq * QTR:(qq + 1) * QTR],
                                             start=(dt == 0), stop=(dt == DTC - 1))
                    yield
                for m in range(QH):
                    _rope(nc, ropet, qrot[m], pqa[m], cos_sb, sin_sb, cols, BF16, ALU)
                yield
                # pass B: k and v (reuses the two banks after rope A reads)
                pk = qkvps.tile([128, QTR], F32, tag="qk0", name=f"pk{qtr}")
                pv = qkvps.tile([128, QTR], F32, tag="qk1", name=f"pv{qtr}")
                for j in range(DTC // 2):
                    hst = hst_tiles[(sh, j)]
                    for t2 in range(2):
                        dt = 2 * j + t2
                        st, sp = dt == 0, dt == DTC - 1
                        nc.tensor.matmul(pk, wk_sb[:, dt, :],
                                         hst[:, t2, qq * QTR:(qq + 1) * QTR],
                                         start=st, stop=sp)
                        nc.tensor.matmul(pv, wv_sb[:, dt, :],
                                         hst[:, t2, qq * QTR:(qq + 1) * QTR],
                                         start=st, stop=sp)
                    yield
                _rope(nc, ropet, krot, pk, cos_sb, sin_sb, cols, BF16, ALU)
                nc.scalar.copy(out=vbf[:, cols], in_=pv)
                nc.sync.dma_start_transpose(
                    out=vnat3[:, 4 * qtr:4 * qtr + 4, :], in_=vbf[:, cols])
                yield

            def oproj_tile(t, g):
                o_sb = osb.tile([128, D], BF16, tag="osb")
                for n in range(D // 512):
                    po = pops.tile([128, 512], F32, tag="po", name=f"po{t}_{n}")
                    for hh in range(QH):
                        nc.tensor.matmul(po, aout[hh][:, t * 128:(t + 1) * 128],
                                         wo_sb[:, hh, n * 512:(n + 1) * 512],
                                         start=(hh == 0), stop=(hh == QH - 1))
                    on_scalar = (n % 2 == 0) if g == 3 else (n == 0)
                    if on_scalar:
                        nc.scalar.copy(out=o_sb[:, n * 512:(n + 1) * 512], in_=po)
                    else:
                        nc.vector.tensor_copy(out=o_sb[:, n * 512:(n + 1) * 512], in_=po)
                nc.sync.dma_start(out=out3[:, t, :], in_=o_sb)

            def attn_chain(c):
                """Scores^T + exp for q-chunk c, l/PV accumulation for units
                g=c (both heads), then o_proj group c.  Software-pipelined
                per k-tile j: the l/PV matmuls for j trail the score matmul
                for j+1 so the in-order PE queue rarely waits on an exp."""
                offs = pt_offs[c]
                jmax = 4 * c + 3
                o_sb0s = {}

                def emit_h0_oproj(t):
                    # h0 half of o_proj group 3, interleaved into head 1's
                    # score stream (aout[0] is ready; keeps the PE warm)
                    ot = osb0p.tile([128, D], BF16, tag="osb0", name=f"osb0_{t}")
                    o_sb0s[t] = ot
                    for n in range(D // 512):
                        po = pops.tile([128, 512], F32, tag="po",
                                       name=f"po0_{t}_{n}")
                        nc.tensor.matmul(po, aout[0][:, t * 128:(t + 1) * 128],
                                         wo_sb[:, 0, n * 512:(n + 1) * 512],
                                         start=True, stop=True)
                        if n % 2 == 0:
                            nc.scalar.copy(out=ot[:, n * 512:(n + 1) * 512], in_=po)
                        else:
                            nc.vector.tensor_copy(out=ot[:, n * 512:(n + 1) * 512],
                                                  in_=po)

                for h in range(QH):
                    l_ps = lpsp.tile([128, 512], F32, tag="l", name=f"l{c}_{h}")
                    pv_ps = pvps.tile([128, 512], F32, tag="pv", name=f"pv{c}_{h}")

                    def lpv(j):
                        off, w = offs[j]
                        co = 512 - w
                        mv = pt[h][c][:, off:off + w]
                        nc.tensor.matmul(l_ps[:, co:512], ones_sb, mv,
                                         start=(j == 0), stop=(j == jmax))
                        nc.tensor.matmul(pv_ps[:, co:512],
                                         vnat[:, j * 128:(j + 1) * 128], mv,
                                         start=(j == 0), stop=(j == jmax))

                    LAG = 3
                    for j in range(jmax + 1):
                        off, w = offs[j]
                        qlo = max(512 * c, 128 * j)
                        s_ch = sps.tile([128, 512], F32, tag="s")
                        nc.tensor.matmul(s_ch[:, 0:w],
                                         krot[:, j * 128:(j + 1) * 128],
                                         qrot[h][:, qlo:qlo + w],
                                         start=True, stop=True)
                        if j >= 4 * c:   # diagonal block: first 128 cols
                            nc.vector.tensor_tensor(
                                out=s_ch[:, 0:128], in0=s_ch[:, 0:128],
                                in1=tri_sb, op=ALU.add)
                        if j >= LAG:
                            lpv(j - LAG)
                        if c == 3 and h == 1 and j % 3 == 2 and j // 3 < 4:
                            emit_h0_oproj(12 + j // 3)
                        nc.scalar.activation(out=pt[h][c][:, off:off + w],
                                             in_=s_ch[:, 0:w],
                                             func=AF.Exp, scale=SCALE)
                        yield
                    for j in range(max(0, jmax + 1 - LAG), jmax + 1):
                        lpv(j)
                    # 1/l = exp(-ln l): two scalar-engine ops in the broadcast
                    # orientation (a DVE reciprocal would cost ~6.5ns/column)
                    lnl = lbp.tile([128, 512], F32, tag="lnl", name=f"lnl{c}_{h}")
                    linv = lbp.tile([128, 512], F32, tag="linv", name=f"linv{c}_{h}")
                    nc.scalar.activation(out=lnl, in_=l_ps, func=AF.Ln)
                    nc.scalar.activation(out=linv, in_=lnl, func=AF.Exp, scale=-1.0)
                    nc.vector.tensor_tensor(
                        out=aout[h][:, c * 512:(c + 1) * 512],
                        in0=pv_ps, in1=linv, op=ALU.mult)
                    yield
                if c < 3:
                    for t in range(4 * c, 4 * c + 4):
                        oproj_tile(t, c)
                        yield
                else:
                    # group 3 second half: h1 matmuls + add-drain + store
                    # (h0 partials were issued right after unit (3,0)'s drain,
                    # so only half the o_proj work sits in the cold tail)
                    for t in range(12, 16):
                        o_sb = osb.tile([128, D], BF16, tag="osb")
                        for n in range(D // 512):
                            po = pops.tile([128, 512], F32, tag="po",
                                           name=f"po1_{t}_{n}")
                            nc.tensor.matmul(po, aout[1][:, t * 128:(t + 1) * 128],
                                             wo_sb[:, 1, n * 512:(n + 1) * 512],
                                             start=True, stop=True)
                            nc.vector.tensor_tensor(
                                out=o_sb[:, n * 512:(n + 1) * 512],
                                in0=po, in1=o_sb0s[t][:, n * 512:(n + 1) * 512],
                                op=ALU.add)
                        nc.sync.dma_start(out=out3[:, t, :], in_=o_sb)
                        yield

            def run_full(gen):
                for _ in gen:
                    pass

            # master schedule
            run_full(qkv_gen(0))
            for q in range(1, 4):
                ga, gb = qkv_gen(q), attn_chain(q - 1)
                na = 18                       # qkv yields per quarter
                nb = 2 * (4 * (q - 1) + 5) + 4  # attn yields for chunk q-1
                ia = ib = 0
                da = db = False
                while not (da and db):
                    if not da and (db or ia * nb <= ib * na):
                        try:
                            next(ga)
                            ia += 1
                        except StopIteration:
                            da = True
                    else:
                        try:
                            next(gb)
                            ib += 1
                        except StopIteration:
                            db = True
            run_full(attn_chain(3))

    nc.compile()
    return nc


def _pm(x):
    """[n*128, M] row-major -> partition-major [128, n*M]."""
    n = x.shape[0] // 128
    return np.ascontiguousarray(
        x.reshape(n, 128, x.shape[1]).transpose(1, 0, 2).reshape(128, -1))


def prep_in_maps(hidden_states, position_ids, Wq, Wk, Wv, Wo):
    import ml_dtypes
    BF = ml_dtypes.bfloat16
    hs = np.asarray(hidden_states, dtype=np.float32).reshape(S, D)
    hsT_pm = _pm(np.ascontiguousarray(hs.T)).astype(BF)             # [128, DTC*S]

    pos = np.asarray(position_ids).reshape(S).astype(np.float32)
    inv = (ROPE_BASE ** (-np.arange(0, HD, 2, dtype=np.float32) / HD))  # [64]
    ang = np.concatenate([pos[None, :] * inv[:, None]] * 2, axis=0)     # [128, S]
    cos_t = np.cos(ang).astype(BF)
    sin_t = np.sin(ang).astype(np.float32)
    sin_signed = np.concatenate([-sin_t[:64], sin_t[64:]], axis=0).astype(BF)

    # transposed diagonal mask: triT[k, q] = 0 where q >= k else NEG
    q_idx = np.arange(128)[None, :]
    k_idx = np.arange(128)[:, None]
    triT = np.where(q_idx >= k_idx, 0.0, NEG).astype(np.float32)

    Wq = np.asarray(Wq, np.float32)
    Wk = np.asarray(Wk, np.float32)
    Wv = np.asarray(Wv, np.float32)
    Wo = np.asarray(Wo, np.float32)

    in_maps = []
    for c in range(NCORES):
        g = (c * QH) // (H // KV)          # kv head owned by this core
        wq_c = Wq[c * QH * 128:(c + 1) * QH * 128]      # [256, D]
        wk_c = Wk[g * 128:(g + 1) * 128]                # [128, D]
        wv_c = Wv[g * 128:(g + 1) * 128]                # [128, D]
        wo_c = Wo[:, c * QH * 128:(c + 1) * QH * 128]   # [D, 256]
        in_maps.append({
            "hs": hsT_pm,
            "wq": _pm(np.ascontiguousarray(wq_c.T)).astype(BF),
            "wk": _pm(np.ascontiguousarray(wk_c.T)).astype(BF),
            "wv": _pm(np.ascontiguousarray(wv_c.T)).astype(BF),
            "wo": _pm(np.ascontiguousarray(wo_c.T)).astype(BF),
            "cos": cos_t,
            "sin": sin_signed,
            "tri": triT,
        })
    return in_maps


def combine_outputs(results):
    total = np.zeros((S, D), np.float32)
    for r in results:
        o = np.asarray(r["out"], np.float32)
        total += o.reshape(128, NT, D).transpose(1, 0, 2).reshape(S, D)
    return total[None]


def kernel(hidden_states, attention_mask, position_ids, Wq, Wk, Wv, Wo):
    from concourse import bass_utils
    if "nc" not in _CACHE:
        _CACHE["nc"] = build_nc()
    nc = _CACHE["nc"]
    in_maps = prep_in_maps(hidden_states, position_ids, Wq, Wk, Wv, Wo)
    res = bass_utils.run_bass_kernel_spmd(nc, in_maps, core_ids=list(range(NCORES)))
    return combine_outputs(res.results)


# revision 37
# speedup vs baseline: 1.0113x; 1.0113x over previous
"""LlamaAttention (B=1, S=2048, D=2048, H=16, KV=4) on 8 TRN2 NeuronCores.

Tensor-parallel over heads: core c owns q-heads [2c, 2c+1] and kv-head c//2.
Each core computes partial = attn_out_c @ Wo[:, c-slice].T over the full
sequence; the all-reduce after o_proj happens on the host (sum of partials).

Layout strategy: everything on-chip lives feature-on-partitions ("transposed"):
  hsT [d, s], qT/kT/vT [hd, s], attn_outT [hd, s].  The host pre-transposes
hidden_states and weights into partition-major [128, N] bf16 arrays so every
DMA is contiguous; rope tables (bf16 cos / sign-adjusted sin) and the causal
diagonal mask block (transposed) are precomputed on host.

Key design point vs a classic flash-style schedule: attention scores are
computed TRANSPOSED (s^T[k, q] via stationary = krot k-tile, moving = qrot
q-columns), so the exp writes P^T [k-on-partitions] straight into SBUF and
P@V consumes it directly as the moving operand.  No DMA xbar transposes of P
are needed (each DMA transpose acts as a full DMA-subsystem barrier on TRN2,
which serialized the previous design).  The softmax row sums l[q] are
accumulated with an all-ones stationary matmul into a PSUM tile alongside
the P@V accumulation (every output partition holds the same l row), so the
aout drain is one reciprocal + one multiply, with no cross-partition
broadcast.

Schedule (all matmuls bf16):
 - QKV projects per quarter in TWO 2-bank passes (q-heads, then k/v), so
   PSUM fits: 2 qkv + 2 scores + 1 l + 1 pv + 2 o_proj = 8 banks.
 - attention chunk c (scores c for all k-tiles j<=4c+3, l+PV for units
   g=c, o_proj group c) interleaves into quarter c+1's projection stream;
   chunk 3 forms the tail as a per-j software pipeline
   [score MM j+1 | exp j | l/PV MM j] so the PE never waits a full exp.
 - softmax: plain exp (no running max: scores are O(6) sigma so fp32 exp
   cannot overflow); P^T stays unnormalized bf16, aout = pv * (1/l).
 - PSUM drains: rope uses one scalar cast then all-bf16 DVE ops (the
   rotate-half is a partition-shifted DVE copy); o_proj casts alternate
   scalar/vector.  Output partials are bf16, host all-reduces in f32.
"""
import math
import numpy as np

S = 2048
D = 2048
HD = 128
H = 16
KV = 4
NCORES = 8
NT = S // 128          # 16 sequence tiles
DTC = D // 128         # 16 feature chunks
QH = H // NCORES       # 2 q-heads per core
ROPE_BASE = 10000.0
SCALE = 1.0 / math.sqrt(HD)
NEG = -1.0e9

_CACHE = {}


def _pt_layout(c):
    """Column layout of the P^T chunk-c buffer: per k-tile j the slice
    (offset, width) covering q-columns [max(512c, 128j), 512c+512)."""
    offs = []
    off = 0
    for j in range(4 * c + 4):
        w = min(512, 512 * c + 512 - 128 * j)
        offs.append((off, w))
        off += w
    return offs, off


def _rope(nc, pool, dst, src_ps, cos_sb, sin_sb, cols, BF16, ALU):
    """dst[:, cols] = src*cos + rotate_half(src)*sin  (src: psum [128, w])."""
    w = cols.stop - cols.start
    raw = pool.tile([128, w], BF16, tag="roperaw")
    rot = pool.tile([128, w], BF16, tag="roperot")
    t1 = pool.tile([128, w], BF16, tag="ropet1")
    nc.scalar.copy(out=raw, in_=src_ps)
    nc.vector.tensor_copy(out=rot[0:64, :], in_=raw[64:128, :])
    nc.vector.tensor_copy(out=rot[64:128, :], in_=raw[0:64, :])
    nc.vector.tensor_tensor(out=t1, in0=raw, in1=cos_sb[:, cols], op=ALU.mult)
    nc.vector.tensor_tensor(out=rot, in0=rot, in1=sin_sb[:, cols], op=ALU.mult)
    nc.vector.tensor_tensor(out=dst[:, cols], in0=t1, in1=rot, op=ALU.add)


def build_nc():
    import concourse.bacc as bacc
    import concourse.tile as tile
    from concourse import mybir

    F32 = mybir.dt.float32
    BF16 = mybir.dt.bfloat16
    AF = mybir.ActivationFunctionType
    ALU = mybir.AluOpType

    nc = bacc.Bacc("TRN2", target_bir_lowering=False, debug=False)
    hs_d = nc.dram_tensor("hs", [128, DTC * S], BF16, kind="ExternalInput").ap()
    wq_d = nc.dram_tensor("wq", [128, DTC * QH * 128], BF16, kind="ExternalInput").ap()
    wk_d = nc.dram_tensor("wk", [128, DTC * 128], BF16, kind="ExternalInput").ap()
    wv_d = nc.dram_tensor("wv", [128, DTC * 128], BF16, kind="ExternalInput").ap()
    wo_d = nc.dram_tensor("wo", [128, QH * D], BF16, kind="ExternalInput").ap()
    cos_d = nc.dram_tensor("cos", [128, S], BF16, kind="ExternalInput").ap()
    sin_d = nc.dram_tensor("sin", [128, S], BF16, kind="ExternalInput").ap()
    tri_d = nc.dram_tensor("tri", [128, 128], F32, kind="ExternalInput").ap()
    out_d = nc.dram_tensor("out", [128, NT * D], BF16, kind="ExternalOutput").ap()

    hs3 = hs_d.rearrange("p (t s) -> p t s", t=DTC)
    out3 = out_d.rearrange("p (t d) -> p t d", t=NT)

    HALF = S // 2
    QTR = S // 4

    with tile.TileContext(nc) as tc:
        with tc.tile_pool(name="consts", bufs=1) as consts, \
             tc.tile_pool(name="persist", bufs=1) as persist, \
             tc.tile_pool(name="hsp", bufs=8) as hsp, \
             tc.tile_pool(name="ropet", bufs=1) as ropet, \
             tc.tile_pool(name="lbp", bufs=2) as lbp, \
             tc.tile_pool(name="osb", bufs=4) as osb, \
             tc.tile_pool(name="osb0", bufs=4) as osb0p, \
             tc.tile_pool(name="qkvps", bufs=1, space="PSUM") as qkvps, \
             tc.tile_pool(name="sps", bufs=2, space="PSUM") as sps, \
             tc.tile_pool(name="lps", bufs=1, space="PSUM") as lpsp, \
             tc.tile_pool(name="pvps", bufs=1, space="PSUM") as pvps, \
             tc.tile_pool(name="pops", bufs=2, space="PSUM") as pops:
            tri_sb = consts.tile([128, 128], F32)
            ones_sb = consts.tile([128, 128], BF16)
            cos_sb = consts.tile([128, S], BF16)
            sin_sb = consts.tile([128, S], BF16)
            wq_sb = consts.tile([128, DTC, QH * 128], BF16)
            wk_sb = consts.tile([128, DTC, 128], BF16)
            wv_sb = consts.tile([128, DTC, 128], BF16)
            wo_sb = consts.tile([128, QH, D], BF16)

            qrot = [persist.tile([128, S], BF16, tag=f"qrot{h}", name=f"qrot{h}") for h in range(QH)]
            krot = persist.tile([128, S], BF16, tag="krot")
            vbf = persist.tile([128, S], BF16, tag="vbf")
            vnat = persist.tile([128, NT * 128], BF16, tag="vnat")
            vnat3 = vnat.rearrange("p (t f) -> p t f", t=NT)
            aout = [persist.tile([128, S], BF16, tag=f"aout{h}", name=f"aout{h}") for h in range(QH)]
            # P^T chunk buffers: pt[h][c] holds exp(s^T) for q-chunk c,
            # k-tiles j=0..4c+3 consecutively (see _pt_layout).
            pt_offs = {}
            pt = [[None] * 4 for _ in range(QH)]
            for h in range(QH):
                for c in range(4):
                    offs, L = _pt_layout(c)
                    pt_offs[c] = offs
                    pt[h][c] = persist.tile([128, L], BF16, tag=f"pt{h}_{c}",
                                            name=f"pt{h}_{c}")

            nc.vector.memset(ones_sb, 1.0)

            hst_tiles = {}

            def load_hst(sh, j):
                t = hsp.tile([128, 2, HALF], BF16, tag="hst", name=f"hst{sh}_{j}")
                nc.sync.dma_start(
                    out=t, in_=hs3[:, 2 * j:2 * j + 2, sh * HALF:(sh + 1) * HALF])
                hst_tiles[(sh, j)] = t
                return t

            # DMA prologue (all on the sync ring): first weight chunks + hs
            # tiles ahead of the bulk so the PE starts within ~3us.
            wq3 = wq_d.rearrange("p (t m) -> p t m", t=DTC)
            wk3 = wk_d.rearrange("p (t m) -> p t m", t=DTC)
            wv3 = wv_d.rearrange("p (t m) -> p t m", t=DTC)

            # pass A consumes only wq, so stream all of wq first, then wk/wv
            # (needed ~8us later by pass B), interleaved with hs tiles.
            nc.sync.dma_start(out=tri_sb, in_=tri_d)
            nc.sync.dma_start(out=wq_sb[:, 0:4, :], in_=wq3[:, 0:4, :])
            load_hst(0, 0)
            load_hst(0, 1)
            nc.sync.dma_start(out=wk_sb, in_=wk3)
            nc.sync.dma_start(out=wv_sb, in_=wv3)
            nc.sync.dma_start(out=wq_sb[:, 4:10, :], in_=wq3[:, 4:10, :])
            load_hst(0, 2)
            load_hst(0, 3)
            nc.sync.dma_start(out=wq_sb[:, 10:16, :], in_=wq3[:, 10:16, :])
            load_hst(0, 4)
            load_hst(0, 5)
            nc.sync.dma_start(out=cos_sb, in_=cos_d)
            load_hst(0, 6)
            load_hst(0, 7)
            nc.sync.dma_start(out=sin_sb, in_=sin_d)
            nc.sync.dma_start(out=wo_sb, in_=wo_d.rearrange("p (h m) -> p h m", h=QH))

            # ---------------- generators --------------------------------
            def qkv_gen(qtr):
                """Quarter qtr of the QKV projection in two 2-bank passes."""
                sh, qq = divmod(qtr, 2)
                cols = slice(qtr * QTR, (qtr + 1) * QTR)
                if qtr == 1:
                    for j in range(8):   # prefetch half 1 as slots free up
                        load_hst(1, j)
                # pass A: the two q heads
                pqa = [qkvps.tile([128, QTR], F32, tag=f"qk{m}", name=f"pqa{qtr}_{m}")
                       for m in range(QH)]
                for j in range(DTC // 2):
                    hst = hst_tiles[(sh, j)]
                    for t2 in range(2):
                        dt = 2 * j + t2
                        for m in range(QH):
                            nc.tensor.matmul(pqa[m], wq_sb[:, dt, m * 128:(m + 1) * 128],
                                             hst[:, t2, qq * QTR:(qq + 1) * QTR],
                                             start=(dt == 0), stop=(dt == DTC - 1))
                    yield
                for m in range(QH):
                    _rope(nc, ropet, qrot[m], pqa[m], cos_sb, sin_sb, cols, BF16, ALU)
                yield
                # pass B: k and v (reuses the two banks after rope A reads)
                pk = qkvps.tile([128, QTR], F32, tag="qk0", name=f"pk{qtr}")
                pv = qkvps.tile([128, QTR], F32, tag="qk1", name=f"pv{qtr}")
                for j in range(DTC // 2):
                    hst = hst_tiles[(sh, j)]
                    for t2 in range(2):
                        dt = 2 * j + t2
                        st, sp = dt == 0, dt == DTC - 1
                        nc.tensor.matmul(pk, wk_sb[:, dt, :],
                                         hst[:, t2, qq * QTR:(qq + 1) * QTR],
                                         start=st, stop=sp)
                        nc.tensor.matmul(pv, wv_sb[:, dt, :],
                                         hst[:, t2, qq * QTR:(qq + 1) * QTR],
                                         start=st, stop=sp)
                    yield
                _rope(nc, ropet, krot, pk, cos_sb, sin_sb, cols, BF16, ALU)
                nc.scalar.copy(out=vbf[:, cols], in_=pv)
                nc.sync.dma_start_transpose(
                    out=vnat3[:, 4 * qtr:4 * qtr + 4, :], in_=vbf[:, cols])
                yield

            def oproj_tile(t, g):
                o_sb = osb.tile([128, D], BF16, tag="osb")
                for n in range(D // 512):
                    po = pops.tile([128, 512], F32, tag="po", name=f"po{t}_{n}")
                    for hh in range(QH):
                        nc.tensor.matmul(po, aout[hh][:, t * 128:(t + 1) * 128],
                                         wo_sb[:, hh, n * 512:(n + 1) * 512],
                                         start=(hh == 0), stop=(hh == QH - 1))
                    on_scalar = (n % 2 == 0) if g == 3 else (n == 0)
                    if on_scalar:
                        nc.scalar.copy(out=o_sb[:, n * 512:(n + 1) * 512], in_=po)
                    else:
                        nc.vector.tensor_copy(out=o_sb[:, n * 512:(n + 1) * 512], in_=po)
                nc.sync.dma_start(out=out3[:, t, :], in_=o_sb)

            def attn_chain(c):
                """Scores^T + exp for q-chunk c, l/PV accumulation for units
                g=c (both heads), then o_proj group c.  Software-pipelined
                per k-tile j: the l/PV matmuls for j trail the score matmul
                for j+1 so the in-order PE queue rarely waits on an exp."""
                offs = pt_offs[c]
                jmax = 4 * c + 3
                o_sb0s = {}

                def emit_h0_oproj(t):
                    # h0 half of o_proj group 3, interleaved into head 1's
                    # score stream (aout[0] is ready; keeps the PE warm)
                    ot = osb0p.tile([128, D], BF16, tag="osb0", name=f"osb0_{t}")
                    o_sb0s[t] = ot
                    for n in range(D // 512):
                        po = pops.tile([128, 512], F32, tag="po",
                                       name=f"po0_{t}_{n}")
                        nc.tensor.matmul(po, aout[0][:, t * 128:(t + 1) * 128],
                                         wo_sb[:, 0, n * 512:(n + 1) * 512],
                                         start=True, stop=True)
                        nc.vector.tensor_copy(out=ot[:, n * 512:(n + 1) * 512],
                                              in_=po)

                for h in range(QH):
                    l_ps = lpsp.tile([128, 512], F32, tag="l", name=f"l{c}_{h}")
                    pv_ps = pvps.tile([128, 512], F32, tag="pv", name=f"pv{c}_{h}")

                    def lpv(j):
                        off, w = offs[j]
                        co = 512 - w
                        mv = pt[h][c][:, off:off + w]
                        nc.tensor.matmul(l_ps[:, co:512], ones_sb, mv,
                                         start=(j == 0), stop=(j == jmax))
                        nc.tensor.matmul(pv_ps[:, co:512],
                                         vnat[:, j * 128:(j + 1) * 128], mv,
                                         start=(j == 0), stop=(j == jmax))

                    LAG = 3
                    for j in range(jmax + 1):
                        off, w = offs[j]
                        qlo = max(512 * c, 128 * j)
                        s_ch = sps.tile([128, 512], F32, tag="s")
                        nc.tensor.matmul(s_ch[:, 0:w],
                                         krot[:, j * 128:(j + 1) * 128],
                                         qrot[h][:, qlo:qlo + w],
                                         start=True, stop=True)
                        if j >= 4 * c:   # diagonal block: first 128 cols
                            nc.vector.tensor_tensor(
                                out=s_ch[:, 0:128], in0=s_ch[:, 0:128],
                                in1=tri_sb, op=ALU.add)
                        if j >= LAG:
                            lpv(j - LAG)
                        if c == 3 and h == 1 and j % 3 == 2 and j // 3 < 4:
                            emit_h0_oproj(12 + j // 3)
                        nc.scalar.activation(out=pt[h][c][:, off:off + w],
                                             in_=s_ch[:, 0:w],
                                             func=AF.Exp, scale=SCALE)
                        yield
                    for j in range(max(0, jmax + 1 - LAG), jmax + 1):
                        lpv(j)
                    # lazy normalization: free both psum banks fast (DVE cast
                    # for pv, scalar Ln for l, concurrently), then multiply by
                    # 1/l = exp(-ln l) off the critical path (a DVE reciprocal
                    # would cost ~6.5ns/column).
                    aout_u = lbp.tile([128, 512], BF16, tag="aoutu",
                                      name=f"aoutu{c}_{h}")
                    lnl = lbp.tile([128, 512], F32, tag="lnl", name=f"lnl{c}_{h}")
                    linv = lbp.tile([128, 512], F32, tag="linv", name=f"linv{c}_{h}")
                    nc.vector.tensor_copy(out=aout_u, in_=pv_ps)
                    nc.scalar.activation(out=lnl, in_=l_ps, func=AF.Ln)
                    nc.scalar.activation(out=linv, in_=lnl, func=AF.Exp, scale=-1.0)
                    nc.vector.tensor_tensor(
                        out=aout[h][:, c * 512:(c + 1) * 512],
                        in0=aout_u, in1=linv, op=ALU.mult)
                    yield
                if c < 3:
                    for t in range(4 * c, 4 * c + 4):
                        oproj_tile(t, c)
                        yield
                else:
                    # group 3 second half: h1 matmuls + add-drain + store
                    # (h0 partials were issued right after unit (3,0)'s drain,
                    # so only half the o_proj work sits in the cold tail)
                    for t in range(12, 16):
                        o_sb = osb.tile([128, D], BF16, tag="osb")
                        for n in range(D // 512):
                            po = pops.tile([128, 512], F32, tag="po",
                                           name=f"po1_{t}_{n}")
                            nc.tensor.matmul(po, aout[1][:, t * 128:(t + 1) * 128],
                                             wo_sb[:, 1, n * 512:(n + 1) * 512],
                                             start=True, stop=True)
                            nc.vector.tensor_tensor(
                                out=o_sb[:, n * 512:(n + 1) * 512],
                                in0=po, in1=o_sb0s[t][:, n * 512:(n + 1) * 512],
                                op=ALU.add)
                        nc.sync.dma_start(out=out3[:, t, :], in_=o_sb)
                        yield

            def run_full(gen):
                for _ in gen:
                    pass

            # master schedule
            run_full(qkv_gen(0))
            for q in range(1, 4):
                ga, gb = qkv_gen(q), attn_chain(q - 1)
                na = 18                       # qkv yields per quarter
                nb = 2 * (4 * (q - 1) + 5) + 4  # attn yields for chunk q-1
                ia = ib = 0
                da = db = False
                while not (da and db):
                    if not da and (db or ia * nb <= ib * na):
                        try:
                            next(ga)
                            ia += 1
                        except StopIteration:
                            da = True
                    else:
                        try:
                            next(gb)
                            ib += 1
                        except StopIteration:
                            db = True
            run_full(attn_chain(3))

    nc.compile()
    return nc


def _pm(x):
    """[n*128, M] row-major -> partition-major [128, n*M]."""
    n = x.shape[0] // 128
    return np.ascontiguousarray(
        x.reshape(n, 128, x.shape[1]).transpose(1, 0, 2).reshape(128, -1))


def prep_in_maps(hidden_states, position_ids, Wq, Wk, Wv, Wo):
    import ml_dtypes
    BF = ml_dtypes.bfloat16
    hs = np.asarray(hidden_states, dtype=np.float32).reshape(S, D)
    hsT_pm = _pm(np.ascontiguousarray(hs.T)).astype(BF)             # [128, DTC*S]

    pos = np.asarray(position_ids).reshape(S).astype(np.float32)
    inv = (ROPE_BASE ** (-np.arange(0, HD, 2, dtype=np.float32) / HD))  # [64]
    ang = np.concatenate([pos[None, :] * inv[:, None]] * 2, axis=0)     # [128, S]
    cos_t = np.cos(ang).astype(BF)
    sin_t = np.sin(ang).astype(np.float32)
    sin_signed = np.concatenate([-sin_t[:64], sin_t[64:]], axis=0).astype(BF)

    # transposed diagonal mask: triT[k, q] = 0 where q >= k else NEG
    q_idx = np.arange(128)[None, :]
    k_idx = np.arange(128)[:, None]
    triT = np.where(q_idx >= k_idx, 0.0, NEG).astype(np.float32)

    Wq = np.asarray(Wq, np.float32)
    Wk = np.asarray(Wk, np.float32)
    Wv = np.asarray(Wv, np.float32)
    Wo = np.asarray(Wo, np.float32)

    in_maps = []
    for c in range(NCORES):
        g = (c * QH) // (H // KV)          # kv head owned by this core
        wq_c = Wq[c * QH * 128:(c + 1) * QH * 128]      # [256, D]
        wk_c = Wk[g * 128:(g + 1) * 128]                # [128, D]
        wv_c = Wv[g * 128:(g + 1) * 128]                # [128, D]
        wo_c = Wo[:, c * QH * 128:(c + 1) * QH * 128]   # [D, 256]
        in_maps.append({
            "hs": hsT_pm,
            "wq": _pm(np.ascontiguousarray(wq_c.T)).astype(BF),
            "wk": _pm(np.ascontiguousarray(wk_c.T)).astype(BF),
            "wv": _pm(np.ascontiguousarray(wv_c.T)).astype(BF),
            "wo": _pm(np.ascontiguousarray(wo_c.T)).astype(BF),
            "cos": cos_t,
            "sin": sin_signed,
            "tri": triT,
        })
    return in_maps


def combine_outputs(results):
    total = np.zeros((S, D), np.float32)
    for r in results:
        o = np.asarray(r["out"], np.float32)
        total += o.reshape(128, NT, D).transpose(1, 0, 2).reshape(S, D)
    return total[None]


def kernel(hidden_states, attention_mask, position_ids, Wq, Wk, Wv, Wo):
    from concourse import bass_utils
    if "nc" not in _CACHE:
        _CACHE["nc"] = build_nc()
    nc = _CACHE["nc"]
    in_maps = prep_in_maps(hidden_states, position_ids, Wq, Wk, Wv, Wo)
    res = bass_utils.run_bass_kernel_spmd(nc, in_maps, core_ids=list(range(NCORES)))
    return combine_outputs(res.results)
